# revision 12
# baseline (speedup 1.0000x reference)
"""Binary conv1d + maxpool + per-channel threshold, Trainium2 Bass kernel.

Problem (hardcoded shapes):
  I:  [64, 64, 16384] f32   -> pad L by (3,3) with -1.0, sign()
  W:  [128, 64, 7]    f32   -> sign()
  conv1d (VALID over padded) -> [64, 128, 16384]
  maxpool1d(k=7, s=2)        -> [64, 128, 8189]
  per-channel threshold      -> +-sign outputs

Sharding: data-parallel over batch, 8 batches per core on 8 cores.

Device algorithm per core (8 batches, as 4 pairs):
  - Binarize to b2 = 2*(x > 0) in {2, 0} (GpSimd chained tensor_scalar),
    so conv_b2 = conv_true + Wsum[co] where Wsum = sum(sign(W)) per out
    channel.  The -Wsum shift folds into the threshold bias.  Activations
    are stored parity-split (even / odd padded columns in separate tiles)
    so matmul rhs slices are contiguous (full-rate bf16 streaming).
  - Conv: 7 accumulating matmuls per output-parity with K=64 contract.
    Batch pair runs CONCURRENTLY on the two PE array halves via row
    tiling (rhs partitions 0:64 / 64:128), even and odd conv columns
    accumulate in separate PSUM tiles.
  - ScalarE does nothing but PSUM->SBUF copies (single ACT function ->
    no activation-table swaps).
  - DVE pool: T[i] = max(ce[i], o[i], ce[i+1]) (2 ops, even widths,
    never in-place, so the 16-bit 2x mode engages), then
    out[l] = max(T[l], T[l+1], T[l+2]) as two batch-wide ops.
  - Threshold: out = ps * (pooled_b2 > tp + Wsum ? 1 : -1) via two
    chained tensor_scalar ops (DVE or GpSimd).
"""

import numpy as np

B, Cin, L = 64, 64, 16384
Cout, K = 128, 7
PAD = 3
LPAD = L + 2 * PAD          # 16390
Lp = (L - 7) // 2 + 1       # 8189
NT = Lp + 3                 # 8192 T-buffer slots (8191 real + 1 garbage)
NCORES = 8
BPC = B // NCORES           # 8 batches per core
PAIRS = BPC // 2            # 4
NPAR = LPAD // 2            # 8195 entries in each parity tile

GROUP = 1024                # conv cols per group (512 even + 512 odd)
GSTRIDE = GROUP - 2
NGROUPS = 16                # cover T[0:8176)
TAIL_S = 16352
TAIL_W = 32
SIGN_CHUNK = 2048

# batches whose threshold ops run on gpsimd (rest on vector)
THRESH_GP_BATCHES = 4

_CACHE = {}


def _build(fast: bool, unit: bool):
    import concourse.mybir as mybir
    from concourse import bacc
    from concourse.tile import TileContext

    f32 = mybir.dt.float32
    bf16 = mybir.dt.bfloat16
    fp16 = mybir.dt.float16
    AF = mybir.ActivationFunctionType
    OP = mybir.AluOpType

    nc = bacc.Bacc()
    I_in = nc.declare_dram_parameter("I", [BPC, Cin, L], f32, isOutput=False)
    W_in = nc.declare_dram_parameter("W", [Cout, Cin, K], f32, isOutput=False)
    thr_in = nc.declare_dram_parameter("thr", [Cout, 8], f32, isOutput=False)
    O_out = nc.declare_dram_parameter("O", [BPC, Cout, Lp], bf16, isOutput=True)

    with TileContext(nc) as tc:
        with (
            tc.tile_pool(name="wpool", bufs=1) as wpool,
            tc.tile_pool(name="spool", bufs=2 if fast else 1) as spool,
            tc.tile_pool(name="fpool", bufs=2) as fpool,
            tc.tile_pool(name="tpool", bufs=2) as tpool,
            tc.tile_pool(name="vpool", bufs=2 if fast else 1) as vpool,
            tc.tile_pool(name="opool", bufs=2) as opool,
            tc.tile_pool(name="gpool", bufs=1) as gpool,
            tc.tile_pool(name="cepool", bufs=4) as cepool,
            tc.tile_pool(name="rpool", bufs=2) as rpool,
            tc.tile_pool(name="pspool", bufs=8, space="PSUM") as pspool,
        ):
            # ---- weight prep: sign(W) as {1,-1} bf16, layout [ci, k*128+co]
            wf = wpool.tile([128, K * Cout], f32, tag="wf")
            w_src = W_in[:].rearrange("co ci k -> ci k co")
            wf_v = wf[:].rearrange("p (k co) -> p k co", co=Cout)
            nc.sync.dma_start(out=wf_v[0:64, :, :], in_=w_src)
            nc.sync.dma_start(out=wf_v[64:128, :, :], in_=w_src)
            wb = wpool.tile([128, K * Cout], bf16, tag="wb")
            # (w > 0)*2 - 1  ->  {1, -1}
            nc.gpsimd.tensor_scalar(out=wb[:, :], in0=wf[:, :], scalar1=0.0,
                                    scalar2=2.0, op0=OP.is_gt, op1=OP.mult)
            nc.gpsimd.tensor_scalar(out=wb[:, :], in0=wb[:, :], scalar1=1.0,
                                    scalar2=None, op0=OP.subtract)

            # ---- Wsum[co] = sum_ci,k wb : 7 matmuls against a ones vector
            ones = wpool.tile([128, 1], bf16, tag="ones")
            nc.vector.memset(ones[:, :], 1.0)
            psw = pspool.tile([128, 512], f32, tag="ps", name="psw")
            for tap in range(K):
                nc.tensor.matmul(
                    psw[:, 0:1], wb[0:64, tap * Cout:(tap + 1) * Cout],
                    ones[0:64, 0:1], start=(tap == 0), stop=(tap == K - 1))

            # ---- thresholds [128, 8] f32; effective biases t' = t + Wsum
            thr = wpool.tile([128, 8], f32, tag="thr")
            nc.sync.dma_start(out=thr[:, :], in_=thr_in[:])
            thrp = wpool.tile([128, 4], f32, tag="thrp")
            # col0: tp + Wsum ; col1: tm + Wsum ; col2: Wsum (for p>=0)
            nc.vector.tensor_tensor(out=thrp[:, 0:1], in0=thr[:, 1:2],
                                    in1=psw[:, 0:1], op=OP.add)
            nc.vector.tensor_tensor(out=thrp[:, 1:2], in0=thr[:, 2:3],
                                    in1=psw[:, 0:1], op=OP.add)
            nc.vector.tensor_scalar(out=thrp[:, 2:3], in0=psw[:, 0:1],
                                    scalar1=0.0, scalar2=None, op0=OP.add)

            groups = [(g * GSTRIDE, GROUP, g * (GROUP // 2 - 1))
                      for g in range(NGROUPS)]
            groups.append((TAIL_S, TAIL_W, NGROUPS * (GROUP // 2 - 1)))

            batch_idx = 0
            for p in range(PAIRS):
                # ---- binarize to {2,0}, parity-split, batch pair stacked
                SEt = spool.tile([128, NPAR], bf16, tag="SE")
                SOt = spool.tile([128, NPAR], bf16, tag="SO")
                for c0 in range(0, L, SIGN_CHUNK):
                    F = fpool.tile([128, SIGN_CHUNK], f32, tag="F")
                    nc.sync.dma_start(
                        out=F[:, :],
                        in_=I_in[2 * p:2 * p + 2, :, c0:c0 + SIGN_CHUNK]
                        .rearrange("b ci l -> (b ci) l"))
                    Fv = F[:].rearrange("p (n two) -> p n two", two=2)
                    half = SIGN_CHUNK // 2
                    # input col i lands at padded col i+3: odd i -> even pad
                    nc.gpsimd.tensor_scalar(
                        out=SEt[:, c0 // 2 + 2:c0 // 2 + 2 + half],
                        in0=Fv[:, :, 1], scalar1=0.0, scalar2=2.0,
                        op0=OP.is_gt, op1=OP.mult)
                    nc.gpsimd.tensor_scalar(
                        out=SOt[:, c0 // 2 + 1:c0 // 2 + 1 + half],
                        in0=Fv[:, :, 0], scalar1=0.0, scalar2=2.0,
                        op0=OP.is_gt, op1=OP.mult)
                # padding -> b2 = 0
                nc.vector.memset(SEt[:, 0:2], 0.0)
                nc.vector.memset(SEt[:, NPAR - 1:NPAR], 0.0)
                nc.vector.memset(SOt[:, 0:1], 0.0)
                nc.vector.memset(SOt[:, NPAR - 2:NPAR], 0.0)

                # ---- conv + fused pool stage 1 into T buffers
                Tlo = tpool.tile([128, NT], fp16, tag="T")
                Thi = tpool.tile([128, NT], fp16, tag="T")

                def rhs(par, half, s, tap, n):
                    # conv col j = s + 2i (+1 if par odd), tap k:
                    # padded[j + k] column index
                    if par == 0:
                        src, n0 = (SEt, (s + tap) // 2) if tap % 2 == 0 \
                            else (SOt, (s + tap - 1) // 2)
                    else:
                        src, n0 = (SOt, (s + tap) // 2) if tap % 2 == 0 \
                            else (SEt, (s + tap + 1) // 2)
                    return src[64 * half:64 * (half + 1), n0:n0 + n]

                for (s, w, t0) in groups:
                    h = w // 2
                    pse = [pspool.tile([128, h], f32, tag="ps",
                                       name=f"pse{i}_{p}_{s}")
                           for i in range(2)]
                    pso = [pspool.tile([128, h], f32, tag="ps",
                                       name=f"pso{i}_{p}_{s}")
                           for i in range(2)]
                    for tap in range(K):
                        st = (tap == 0)
                        sp = (tap == K - 1)
                        for half in range(2):
                            lw = wb[64 * half:64 * (half + 1),
                                    tap * Cout:(tap + 1) * Cout]
                            nc.tensor.matmul(
                                pse[half][:, 0:h], lw, rhs(0, half, s, tap, h),
                                start=st, stop=sp)
                            nc.tensor.matmul(
                                pso[half][:, 0:h], lw, rhs(1, half, s, tap, h),
                                start=st, stop=sp)
                    for (half, Tb) in ((0, Tlo), (1, Thi)):
                        CE = cepool.tile([128, 520], fp16, tag="CE")
                        nc.scalar.activation(out=CE[:, 0:h],
                                             in_=pse[half][:, 0:h],
                                             func=AF.Copy)
                        nc.vector.memset(CE[:, h:h + 2], 0.0)
                        R = rpool.tile([128, 512], fp16, tag="R")
                        nc.vector.tensor_tensor(
                            out=R[:, 0:h], in0=CE[:, 0:h],
                            in1=pso[half][:, 0:h], op=OP.max)
                        nc.vector.tensor_tensor(
                            out=Tb[:, t0:t0 + h], in0=R[:, 0:h],
                            in1=CE[:, 1:h + 1], op=OP.max)

                # ---- pool tail + threshold + store, per batch
                for (b, Tb) in ((2 * p, Tlo), (2 * p + 1, Thi)):
                    teng = (nc.gpsimd if batch_idx < THRESH_GP_BATCHES
                            else nc.vector)
                    batch_idx += 1
                    V = vpool.tile([128, Lp + 1], fp16, tag="V")
                    Ofin = opool.tile([128, Lp + 1], bf16, tag="Ofin")
                    nc.vector.tensor_tensor(out=V[:, 0:Lp + 1],
                                            in0=Tb[:, 0:Lp + 1],
                                            in1=Tb[:, 1:Lp + 2], op=OP.max)
                    nc.vector.tensor_tensor(out=V[:, 0:Lp + 1],
                                            in0=V[:, 0:Lp + 1],
                                            in1=Tb[:, 2:Lp + 3], op=OP.max)
                    if fast:
                        s2 = 2.0 if unit else thr[:, 3:4]
                        s3 = 1.0 if unit else thr[:, 4:5]
                        # {2ps, 0} then -ps -> {ps, -ps}
                        teng.tensor_scalar(
                            out=V[:, :], in0=V[:, :], scalar1=thrp[:, 0:1],
                            scalar2=s2, op0=OP.is_gt, op1=OP.mult)
                        teng.tensor_scalar(
                            out=Ofin[:, :], in0=V[:, :], scalar1=s3,
                            scalar2=None, op0=OP.subtract)
                    else:
                        G = gpool.tile([128, Lp + 1], fp16, tag="G")
                        Gn = gpool.tile([128, Lp + 1], fp16, tag="Gn")
                        G0 = gpool.tile([128, Lp + 1], fp16, tag="G0")
                        # pos branch: {ps, -ps}
                        nc.vector.tensor_scalar(
                            out=G[:, :], in0=V[:, :], scalar1=thrp[:, 0:1],
                            scalar2=thr[:, 3:4], op0=OP.is_gt, op1=OP.mult)
                        nc.vector.tensor_scalar(
                            out=G[:, :], in0=G[:, :], scalar1=thr[:, 4:5],
                            scalar2=None, op0=OP.subtract)
                        # neg branch: {ms, -ms}
                        nc.vector.tensor_scalar(
                            out=Gn[:, :], in0=V[:, :], scalar1=thrp[:, 1:2],
                            scalar2=thr[:, 5:6], op0=OP.is_gt, op1=OP.mult)
                        nc.vector.tensor_scalar(
                            out=Gn[:, :], in0=Gn[:, :], scalar1=thr[:, 6:7],
                            scalar2=None, op0=OP.subtract)
                        # select by p_true >= 0  <=>  p_b2 >= Wsum
                        nc.vector.tensor_scalar(
                            out=G0[:, :], in0=V[:, :], scalar1=thrp[:, 2:3],
                            scalar2=None, op0=OP.is_ge)
                        nc.vector.tensor_tensor(out=G[:, :], in0=G[:, :],
                                                in1=Gn[:, :], op=OP.subtract)
                        nc.vector.tensor_tensor(out=G[:, :], in0=G0[:, :],
                                                in1=G[:, :], op=OP.mult)
                        nc.vector.tensor_tensor(out=Ofin[:, :], in0=G[:, :],
                                                in1=Gn[:, :], op=OP.add)
                    nc.sync.dma_start(out=O_out[b], in_=Ofin[:, 0:Lp])

    nc.compile()
    return nc


def _get_nc(fast, unit):
    key = (fast, unit)
    if key not in _CACHE:
        _CACHE[key] = _build(fast, unit)
    return _CACHE[key]


def kernel(I, W, threshold_plus, threshold_minus, threshold_plus_sign,
           threshold_minus_sign):
    from concourse.bass_utils import run_bass_kernel_spmd

    tp = np.asarray(threshold_plus, dtype=np.float32)
    tm = np.asarray(threshold_minus, dtype=np.float32)
    ps = np.asarray(threshold_plus_sign, dtype=np.float32)
    ms = np.asarray(threshold_minus_sign, dtype=np.float32)
    I = np.ascontiguousarray(np.asarray(I, dtype=np.float32))
    W = np.ascontiguousarray(np.asarray(W, dtype=np.float32))

    fast = np.array_equal(tp, tm) and np.array_equal(ps, ms)
    unit = fast and bool(np.all(ps == 1.0))

    thr = np.zeros((Cout, 8), dtype=np.float32)
    thr[:, 0] = -tp
    thr[:, 1] = tp
    thr[:, 2] = tm
    thr[:, 3] = 2.0 * ps
    thr[:, 4] = ps
    thr[:, 5] = 2.0 * ms
    thr[:, 6] = ms

    nc = _get_nc(fast, unit)
    in_maps = [
        {"I": I[c * BPC:(c + 1) * BPC], "W": W, "thr": thr}
        for c in range(NCORES)
    ]
    res = run_bass_kernel_spmd(nc, in_maps, list(range(NCORES)))
    out = np.concatenate(
        [np.asarray(r["O"]).astype(np.float32) for r in res.results], axis=0)
    return out


# revision 13
# speedup vs baseline: 4.4006x; 4.4006x over previous
"""Binary conv1d + maxpool + per-channel threshold, Trainium2 Bass kernel.

Problem (hardcoded shapes):
  I:  [64, 64, 16384] f32   -> pad L by (3,3) with -1.0, sign()
  W:  [128, 64, 7]    f32   -> sign()
  conv1d (VALID over padded) -> [64, 128, 16384]
  maxpool1d(k=7, s=2)        -> [64, 128, 8189]
  per-channel threshold      -> +-sign outputs

Sharding: data-parallel over batch, 8 batches per core on 8 cores.

Device algorithm per core (8 batches, as 4 pairs):
  - ScalarE binarizes (Sign, +-1 bf16) into parity-split tiles (even /
    odd padded columns separately) so matmul rhs slices are contiguous.
    A batch pair is stacked on the 128 partitions (batch 2p on 0:64,
    2p+1 on 64:128) and one full-width ACT pass covers both.
  - Conv: 7 accumulating matmuls per output-parity, K=64 contract.  The
    two batches run CONCURRENTLY on the two PE array halves via row
    tiling; even and odd conv columns accumulate into separate PSUM
    tiles so pool ops never need two PSUM operands.
  - ScalarE evacuates even conv columns (Copy, PSUM->SBUF bf16).
  - DVE pool stage 1: T[i] = max(ce[i], psum_odd[i], ce[i+1]) as two
    non-in-place even-width tensor_tensor maxes (16-bit 2x mode).
  - Pool tail per batch: out[l] = max(T[l], T[l+1], T[l+2]) (2 DVE ops).
  - Threshold out = ps*sign(pooled - tp): split between ACT (Sign with
    per-channel bias) and DVE (is_gt chain) by a balance knob.
  - GpSimd is intentionally idle: its tensor ops measured ~19 cyc/elem
    AND stall concurrent DVE work via the shared SBUF port lock.
"""

import numpy as np

B, Cin, L = 64, 64, 16384
Cout, K = 128, 7
PAD = 3
LPAD = L + 2 * PAD          # 16390
Lp = (L - 7) // 2 + 1       # 8189
NT = Lp + 3                 # 8192 T-buffer slots (8191 real + 1 garbage)
NCORES = 8
BPC = B // NCORES           # 8 batches per core
PAIRS = BPC // 2            # 4
NPAR = LPAD // 2            # 8195 entries in each parity tile

GROUP = 1024                # conv cols per group (512 even + 512 odd)
GSTRIDE = GROUP - 2
NGROUPS = 16                # cover T[0:8176)
TAIL_S = 16352
TAIL_W = 32
SIGN_CHUNK = 2048

# batches whose threshold runs on ScalarE (Sign+bias); rest on DVE
ACT_THRESH_BATCHES = 4

_CACHE = {}


def _build(fast: bool, unit: bool):
    import concourse.mybir as mybir
    from concourse import bacc
    from concourse.tile import TileContext

    f32 = mybir.dt.float32
    bf16 = mybir.dt.bfloat16
    AF = mybir.ActivationFunctionType
    OP = mybir.AluOpType

    nc = bacc.Bacc()
    I_in = nc.declare_dram_parameter("I", [BPC, Cin, L], f32, isOutput=False)
    W_in = nc.declare_dram_parameter("W", [Cout, Cin, K], f32, isOutput=False)
    thr_in = nc.declare_dram_parameter("thr", [Cout, 8], f32, isOutput=False)
    O_out = nc.declare_dram_parameter("O", [BPC, Cout, Lp], bf16, isOutput=True)

    with TileContext(nc) as tc:
        with (
            tc.tile_pool(name="wpool", bufs=1) as wpool,
            tc.tile_pool(name="spool", bufs=2 if fast else 1) as spool,
            tc.tile_pool(name="fpool", bufs=2) as fpool,
            tc.tile_pool(name="tpool", bufs=2) as tpool,
            tc.tile_pool(name="vpool", bufs=2 if fast else 1) as vpool,
            tc.tile_pool(name="opool", bufs=2) as opool,
            tc.tile_pool(name="gpool", bufs=1) as gpool,
            tc.tile_pool(name="cepool", bufs=4) as cepool,
            tc.tile_pool(name="rpool", bufs=2) as rpool,
            tc.tile_pool(name="pspool", bufs=8, space="PSUM") as pspool,
        ):
            # ---- weight prep: sign(W) as {1,-1} bf16, layout [ci, k*128+co]
            wf = wpool.tile([128, K * Cout], f32, tag="wf")
            w_src = W_in[:].rearrange("co ci k -> ci k co")
            wf_v = wf[:].rearrange("p (k co) -> p k co", co=Cout)
            nc.sync.dma_start(out=wf_v[0:64, :, :], in_=w_src)
            nc.sync.dma_start(out=wf_v[64:128, :, :], in_=w_src)
            wb = wpool.tile([128, K * Cout], bf16, tag="wb")
            nc.scalar.activation(out=wb[:, :], in_=wf[:, :], func=AF.Sign)

            # ---- thresholds [128, 8] f32 (col0 = -tp for ACT bias, etc.)
            thr = wpool.tile([128, 8], f32, tag="thr")
            nc.sync.dma_start(out=thr[:, :], in_=thr_in[:])

            groups = [(g * GSTRIDE, GROUP, g * (GROUP // 2 - 1))
                      for g in range(NGROUPS)]
            groups.append((TAIL_S, TAIL_W, NGROUPS * (GROUP // 2 - 1)))

            batch_idx = 0
            for p in range(PAIRS):
                # ---- binarize +-1, parity-split, batch pair stacked
                SEt = spool.tile([128, NPAR], bf16, tag="SE")
                SOt = spool.tile([128, NPAR], bf16, tag="SO")
                for c0 in range(0, L, SIGN_CHUNK):
                    F = fpool.tile([128, SIGN_CHUNK], f32, tag="F")
                    nc.sync.dma_start(
                        out=F[:, :],
                        in_=I_in[2 * p:2 * p + 2, :, c0:c0 + SIGN_CHUNK]
                        .rearrange("b ci l -> (b ci) l"))
                    Fv = F[:].rearrange("p (n two) -> p n two", two=2)
                    half = SIGN_CHUNK // 2
                    # input col i lands at padded col i+3: odd i -> even pad
                    nc.scalar.activation(
                        out=SEt[:, c0 // 2 + 2:c0 // 2 + 2 + half],
                        in_=Fv[:, :, 1], func=AF.Sign)
                    nc.scalar.activation(
                        out=SOt[:, c0 // 2 + 1:c0 // 2 + 1 + half],
                        in_=Fv[:, :, 0], func=AF.Sign)
                # padding -> -1
                nc.vector.memset(SEt[:, 0:2], -1.0)
                nc.vector.memset(SEt[:, NPAR - 1:NPAR], -1.0)
                nc.vector.memset(SOt[:, 0:1], -1.0)
                nc.vector.memset(SOt[:, NPAR - 2:NPAR], -1.0)

                # ---- conv + fused pool stage 1 into T buffers
                Tlo = tpool.tile([128, NT], bf16, tag="T")
                Thi = tpool.tile([128, NT], bf16, tag="T")

                def rhs(par, half, s, tap, n):
                    # conv col j = s + 2i (+1 if par odd), tap k:
                    # padded[j + k] column index
                    if par == 0:
                        src, n0 = (SEt, (s + tap) // 2) if tap % 2 == 0 \
                            else (SOt, (s + tap - 1) // 2)
                    else:
                        src, n0 = (SOt, (s + tap) // 2) if tap % 2 == 0 \
                            else (SEt, (s + tap + 1) // 2)
                    return src[64 * half:64 * (half + 1), n0:n0 + n]

                for (s, w, t0) in groups:
                    h = w // 2
                    pse = [pspool.tile([128, h], f32, tag="ps",
                                       name=f"pse{i}_{p}_{s}")
                           for i in range(2)]
                    pso = [pspool.tile([128, h], f32, tag="ps",
                                       name=f"pso{i}_{p}_{s}")
                           for i in range(2)]
                    for tap in range(K):
                        st = (tap == 0)
                        sp = (tap == K - 1)
                        for half in range(2):
                            lw = wb[64 * half:64 * (half + 1),
                                    tap * Cout:(tap + 1) * Cout]
                            nc.tensor.matmul(
                                pse[half][:, 0:h], lw, rhs(0, half, s, tap, h),
                                start=st, stop=sp)
                            nc.tensor.matmul(
                                pso[half][:, 0:h], lw, rhs(1, half, s, tap, h),
                                start=st, stop=sp)
                    for (half, Tb) in ((0, Tlo), (1, Thi)):
                        CE = cepool.tile([128, 520], bf16, tag="CE")
                        nc.scalar.activation(out=CE[:, 0:h],
                                             in_=pse[half][:, 0:h],
                                             func=AF.Copy)
                        nc.vector.memset(CE[:, h:h + 2], 0.0)
                        R = rpool.tile([128, 512], bf16, tag="R")
                        nc.vector.tensor_tensor(
                            out=R[:, 0:h], in0=CE[:, 0:h],
                            in1=pso[half][:, 0:h], op=OP.max)
                        nc.vector.tensor_tensor(
                            out=Tb[:, t0:t0 + h], in0=R[:, 0:h],
                            in1=CE[:, 1:h + 1], op=OP.max)

                # ---- pool tail + threshold + store, per batch
                for (b, Tb) in ((2 * p, Tlo), (2 * p + 1, Thi)):
                    on_act = batch_idx < ACT_THRESH_BATCHES
                    batch_idx += 1
                    V = vpool.tile([128, Lp + 1], bf16, tag="V")
                    Ofin = opool.tile([128, Lp + 1], bf16, tag="Ofin")
                    nc.vector.tensor_tensor(out=V[:, 0:Lp + 1],
                                            in0=Tb[:, 0:Lp + 1],
                                            in1=Tb[:, 1:Lp + 2], op=OP.max)
                    nc.vector.tensor_tensor(out=V[:, 0:Lp + 1],
                                            in0=V[:, 0:Lp + 1],
                                            in1=Tb[:, 2:Lp + 3], op=OP.max)
                    if fast:
                        if on_act:
                            # sign(pooled - tp), times ps if needed
                            nc.scalar.activation(out=Ofin[:, :], in_=V[:, :],
                                                 func=AF.Sign,
                                                 bias=thr[:, 0:1])
                            if not unit:
                                nc.vector.tensor_scalar(
                                    out=Ofin[:, :], in0=Ofin[:, :],
                                    scalar1=thr[:, 4:5], scalar2=None,
                                    op0=OP.mult)
                        else:
                            s2 = 2.0 if unit else thr[:, 3:4]
                            s3 = 1.0 if unit else thr[:, 4:5]
                            nc.vector.tensor_scalar(
                                out=V[:, :], in0=V[:, :], scalar1=thr[:, 1:2],
                                scalar2=s2, op0=OP.is_gt, op1=OP.mult)
                            nc.vector.tensor_scalar(
                                out=Ofin[:, :], in0=V[:, :], scalar1=s3,
                                scalar2=None, op0=OP.subtract)
                    else:
                        G = gpool.tile([128, Lp + 1], bf16, tag="G")
                        Gn = gpool.tile([128, Lp + 1], bf16, tag="Gn")
                        G0 = gpool.tile([128, Lp + 1], bf16, tag="G0")
                        # pos branch: {ps, -ps}
                        nc.vector.tensor_scalar(
                            out=G[:, :], in0=V[:, :], scalar1=thr[:, 1:2],
                            scalar2=thr[:, 3:4], op0=OP.is_gt, op1=OP.mult)
                        nc.vector.tensor_scalar(
                            out=G[:, :], in0=G[:, :], scalar1=thr[:, 4:5],
                            scalar2=None, op0=OP.subtract)
                        # neg branch: {ms, -ms}
                        nc.vector.tensor_scalar(
                            out=Gn[:, :], in0=V[:, :], scalar1=thr[:, 2:3],
                            scalar2=thr[:, 5:6], op0=OP.is_gt, op1=OP.mult)
                        nc.vector.tensor_scalar(
                            out=Gn[:, :], in0=Gn[:, :], scalar1=thr[:, 6:7],
                            scalar2=None, op0=OP.subtract)
                        nc.vector.tensor_scalar(
                            out=G0[:, :], in0=V[:, :], scalar1=0.0,
                            scalar2=None, op0=OP.is_ge)
                        nc.vector.tensor_tensor(out=G[:, :], in0=G[:, :],
                                                in1=Gn[:, :], op=OP.subtract)
                        nc.vector.tensor_tensor(out=G[:, :], in0=G0[:, :],
                                                in1=G[:, :], op=OP.mult)
                        nc.vector.tensor_tensor(out=Ofin[:, :], in0=G[:, :],
                                                in1=Gn[:, :], op=OP.add)
                    nc.sync.dma_start(out=O_out[b], in_=Ofin[:, 0:Lp])

    nc.compile()
    return nc


def _get_nc(fast, unit):
    key = (fast, unit)
    if key not in _CACHE:
        _CACHE[key] = _build(fast, unit)
    return _CACHE[key]


def kernel(I, W, threshold_plus, threshold_minus, threshold_plus_sign,
           threshold_minus_sign):
    from concourse.bass_utils import run_bass_kernel_spmd

    tp = np.asarray(threshold_plus, dtype=np.float32)
    tm = np.asarray(threshold_minus, dtype=np.float32)
    ps = np.asarray(threshold_plus_sign, dtype=np.float32)
    ms = np.asarray(threshold_minus_sign, dtype=np.float32)
    I = np.ascontiguousarray(np.asarray(I, dtype=np.float32))
    W = np.ascontiguousarray(np.asarray(W, dtype=np.float32))

    fast = np.array_equal(tp, tm) and np.array_equal(ps, ms)
    unit = fast and bool(np.all(ps == 1.0))

    thr = np.zeros((Cout, 8), dtype=np.float32)
    thr[:, 0] = -tp
    thr[:, 1] = tp
    thr[:, 2] = tm
    thr[:, 3] = 2.0 * ps
    thr[:, 4] = ps
    thr[:, 5] = 2.0 * ms
    thr[:, 6] = ms

    nc = _get_nc(fast, unit)
    in_maps = [
        {"I": I[c * BPC:(c + 1) * BPC], "W": W, "thr": thr}
        for c in range(NCORES)
    ]
    res = run_bass_kernel_spmd(nc, in_maps, list(range(NCORES)))
    out = np.concatenate(
        [np.asarray(r["O"]).astype(np.float32) for r in res.results], axis=0)
    return out


# revision 18
# speedup vs baseline: 5.9710x; 1.3568x over previous
"""Binary conv1d + maxpool + per-channel threshold, Trainium2 Bass kernel.

Problem (hardcoded shapes):
  I:  [64, 64, 16384] f32   -> pad L by (3,3) with -1.0, sign()
  W:  [128, 64, 7]    f32   -> sign()
  conv1d (VALID over padded) -> [64, 128, 16384]
  maxpool1d(k=7, s=2)        -> [64, 128, 8189]
  per-channel threshold      -> +-sign outputs

Sharding: data-parallel over batch, 8 batches per core on 8 cores.

Device algorithm per core (8 batches, as 4 pairs):
  - ScalarE binarizes (Sign, +-1 bf16) into parity-split tiles (even /
    odd padded columns separately) so matmul rhs slices are contiguous.
    A batch pair is stacked on the 128 partitions (batch 2p on 0:64,
    2p+1 on 64:128) and one full-width ACT pass covers both.
  - Conv: 7 accumulating matmuls per output-parity, K=64 contract.  The
    two batches run CONCURRENTLY on the two PE array halves via row
    tiling; even and odd conv columns accumulate into separate PSUM
    tiles so pool ops never need two PSUM operands.
  - ScalarE evacuates even conv columns (Copy, PSUM->SBUF bf16).
  - DVE pool stage 1: T[i] = max(ce[i], psum_odd[i], ce[i+1]) as two
    non-in-place even-width tensor_tensor maxes (16-bit 2x mode).
  - Pool tail per batch: out[l] = max(T[l], T[l+1], T[l+2]) (2 DVE ops).
  - Threshold out = ps*sign(pooled - tp): split between ACT (Sign with
    per-channel bias) and DVE (is_gt chain) by a balance knob.
  - GpSimd is intentionally idle: its tensor ops measured ~19 cyc/elem
    AND stall concurrent DVE work via the shared SBUF port lock.
"""

import numpy as np

B, Cin, L = 64, 64, 16384
Cout, K = 128, 7
PAD = 3
LPAD = L + 2 * PAD          # 16390
Lp = (L - 7) // 2 + 1       # 8189
NT = Lp + 3                 # 8192 T-buffer slots (8191 real + 1 garbage)
NCORES = 8
BPC = B // NCORES           # 8 batches per core
PAIRS = BPC // 2            # 4
NPAR = LPAD // 2            # 8195 entries in each parity tile

GROUP = 1024                # conv cols per group (512 even + 512 odd)
GSTRIDE = GROUP - 2
NGROUPS = 16                # cover T[0:8176)
TAIL_S = 16352
TAIL_W = 32
SIGN_CHUNK = 2048

# batches whose threshold runs on ScalarE (Sign+bias); rest on DVE
ACT_THRESH_BATCHES = 4

_CACHE = {}


def _build(fast: bool, unit: bool):
    import concourse.mybir as mybir
    from concourse import bacc
    from concourse.tile import TileContext

    f32 = mybir.dt.float32
    bf16 = mybir.dt.bfloat16
    AF = mybir.ActivationFunctionType
    OP = mybir.AluOpType

    nc = bacc.Bacc()
    I_in = nc.declare_dram_parameter("I", [BPC, Cin, L], f32, isOutput=False)
    # W is passed host-transposed to [Cin, K, Cout] so this DMA reads
    # long contiguous runs (the natural [Cout, Cin, K] layout degenerates
    # into 4-byte strided descriptors).
    W_in = nc.declare_dram_parameter("W", [Cin, K * Cout], f32,
                                     isOutput=False)
    thr_in = nc.declare_dram_parameter("thr", [Cout, 8], f32, isOutput=False)
    O_out = nc.declare_dram_parameter("O", [BPC, Cout, Lp], bf16, isOutput=True)

    with TileContext(nc) as tc:
        with (
            tc.tile_pool(name="wpool", bufs=1) as wpool,
            tc.tile_pool(name="spool", bufs=2 if fast else 1) as spool,
            tc.tile_pool(name="fpool", bufs=2) as fpool,
            tc.tile_pool(name="tpool", bufs=2) as tpool,
            tc.tile_pool(name="vpool", bufs=2 if fast else 1) as vpool,
            tc.tile_pool(name="opool", bufs=2) as opool,
            tc.tile_pool(name="gpool", bufs=1) as gpool,
            tc.tile_pool(name="cepool", bufs=4) as cepool,
            tc.tile_pool(name="rpool", bufs=2) as rpool,
            tc.tile_pool(name="pspool", bufs=8, space="PSUM") as pspool,
        ):
            # ---- weight prep: sign(W) as {1,-1} bf16, layout [ci, k*128+co]
            wf = wpool.tile([128, K * Cout], f32, tag="wf")
            nc.sync.dma_start(out=wf[0:64, :], in_=W_in[:])
            nc.sync.dma_start(out=wf[64:128, :], in_=W_in[:])
            wb = wpool.tile([128, K * Cout], bf16, tag="wb")
            nc.scalar.activation(out=wb[:, :], in_=wf[:, :], func=AF.Sign)

            # ---- thresholds [128, 8] f32 (col0 = -tp for ACT bias, etc.)
            thr = wpool.tile([128, 8], f32, tag="thr")
            nc.sync.dma_start(out=thr[:, :], in_=thr_in[:])

            groups = [(g * GSTRIDE, GROUP, g * (GROUP // 2 - 1))
                      for g in range(NGROUPS)]
            groups.append((TAIL_S, TAIL_W, NGROUPS * (GROUP // 2 - 1)))

            batch_idx = 0
            for p in range(PAIRS):
                # ---- binarize +-1, parity-split, batch pair stacked
                SEt = spool.tile([128, NPAR], bf16, tag="SE")
                SOt = spool.tile([128, NPAR], bf16, tag="SO")
                for c0 in range(0, L, SIGN_CHUNK):
                    F = fpool.tile([128, SIGN_CHUNK], f32, tag="F")
                    nc.sync.dma_start(
                        out=F[:, :],
                        in_=I_in[2 * p:2 * p + 2, :, c0:c0 + SIGN_CHUNK]
                        .rearrange("b ci l -> (b ci) l"))
                    Fv = F[:].rearrange("p (n two) -> p n two", two=2)
                    half = SIGN_CHUNK // 2
                    # input col i lands at padded col i+3: odd i -> even pad
                    nc.scalar.activation(
                        out=SEt[:, c0 // 2 + 2:c0 // 2 + 2 + half],
                        in_=Fv[:, :, 1], func=AF.Sign)
                    nc.scalar.activation(
                        out=SOt[:, c0 // 2 + 1:c0 // 2 + 1 + half],
                        in_=Fv[:, :, 0], func=AF.Sign)
                # padding -> -1
                nc.vector.memset(SEt[:, 0:2], -1.0)
                nc.vector.memset(SEt[:, NPAR - 1:NPAR], -1.0)
                nc.vector.memset(SOt[:, 0:1], -1.0)
                nc.vector.memset(SOt[:, NPAR - 2:NPAR], -1.0)

                # ---- conv + fused pool stage 1 into T buffers
                Tlo = tpool.tile([128, NT], bf16, tag="T")
                Thi = tpool.tile([128, NT], bf16, tag="T")

                def rhs(par, half, s, tap, n):
                    # conv col j = s + 2i (+1 if par odd), tap k:
                    # padded[j + k] column index
                    if par == 0:
                        src, n0 = (SEt, (s + tap) // 2) if tap % 2 == 0 \
                            else (SOt, (s + tap - 1) // 2)
                    else:
                        src, n0 = (SOt, (s + tap) // 2) if tap % 2 == 0 \
                            else (SEt, (s + tap + 1) // 2)
                    return src[64 * half:64 * (half + 1), n0:n0 + n]

                for (s, w, t0) in groups:
                    h = w // 2
                    pse = [pspool.tile([128, h], f32, tag="ps",
                                       name=f"pse{i}_{p}_{s}")
                           for i in range(2)]
                    pso = [pspool.tile([128, h], f32, tag="ps",
                                       name=f"pso{i}_{p}_{s}")
                           for i in range(2)]
                    for tap in range(K):
                        st = (tap == 0)
                        sp = (tap == K - 1)
                        for half in range(2):
                            lw = wb[64 * half:64 * (half + 1),
                                    tap * Cout:(tap + 1) * Cout]
                            nc.tensor.matmul(
                                pse[half][:, 0:h], lw, rhs(0, half, s, tap, h),
                                start=st, stop=sp)
                            nc.tensor.matmul(
                                pso[half][:, 0:h], lw, rhs(1, half, s, tap, h),
                                start=st, stop=sp)
                    for (half, Tb) in ((0, Tlo), (1, Thi)):
                        CE = cepool.tile([128, 520], bf16, tag="CE")
                        nc.scalar.activation(out=CE[:, 0:h],
                                             in_=pse[half][:, 0:h],
                                             func=AF.Copy)
                        nc.vector.memset(CE[:, h:h + 2], 0.0)
                        R = rpool.tile([128, 512], bf16, tag="R")
                        nc.vector.tensor_tensor(
                            out=R[:, 0:h], in0=CE[:, 0:h],
                            in1=pso[half][:, 0:h], op=OP.max)
                        nc.vector.tensor_tensor(
                            out=Tb[:, t0:t0 + h], in0=R[:, 0:h],
                            in1=CE[:, 1:h + 1], op=OP.max)

                # ---- pool tail + threshold + store, per batch
                for (b, Tb) in ((2 * p, Tlo), (2 * p + 1, Thi)):
                    on_act = batch_idx >= (BPC - ACT_THRESH_BATCHES)
                    batch_idx += 1
                    V = vpool.tile([128, Lp + 1], bf16, tag="V")
                    Ofin = opool.tile([128, Lp + 1], bf16, tag="Ofin")
                    nc.vector.tensor_tensor(out=V[:, 0:Lp + 1],
                                            in0=Tb[:, 0:Lp + 1],
                                            in1=Tb[:, 1:Lp + 2], op=OP.max)
                    nc.vector.tensor_tensor(out=V[:, 0:Lp + 1],
                                            in0=V[:, 0:Lp + 1],
                                            in1=Tb[:, 2:Lp + 3], op=OP.max)
                    if fast:
                        if on_act:
                            # sign(pooled - tp), times ps if needed
                            nc.scalar.activation(out=Ofin[:, :], in_=V[:, :],
                                                 func=AF.Sign,
                                                 bias=thr[:, 0:1])
                            if not unit:
                                nc.vector.tensor_scalar(
                                    out=Ofin[:, :], in0=Ofin[:, :],
                                    scalar1=thr[:, 4:5], scalar2=None,
                                    op0=OP.mult)
                        else:
                            s2 = 2.0 if unit else thr[:, 3:4]
                            s3 = 1.0 if unit else thr[:, 4:5]
                            nc.vector.tensor_scalar(
                                out=V[:, :], in0=V[:, :], scalar1=thr[:, 1:2],
                                scalar2=s2, op0=OP.is_gt, op1=OP.mult)
                            nc.vector.tensor_scalar(
                                out=Ofin[:, :], in0=V[:, :], scalar1=s3,
                                scalar2=None, op0=OP.subtract)
                    else:
                        G = gpool.tile([128, Lp + 1], bf16, tag="G")
                        Gn = gpool.tile([128, Lp + 1], bf16, tag="Gn")
                        G0 = gpool.tile([128, Lp + 1], bf16, tag="G0")
                        # pos branch: {ps, -ps}
                        nc.vector.tensor_scalar(
                            out=G[:, :], in0=V[:, :], scalar1=thr[:, 1:2],
                            scalar2=thr[:, 3:4], op0=OP.is_gt, op1=OP.mult)
                        nc.vector.tensor_scalar(
                            out=G[:, :], in0=G[:, :], scalar1=thr[:, 4:5],
                            scalar2=None, op0=OP.subtract)
                        # neg branch: {ms, -ms}
                        nc.vector.tensor_scalar(
                            out=Gn[:, :], in0=V[:, :], scalar1=thr[:, 2:3],
                            scalar2=thr[:, 5:6], op0=OP.is_gt, op1=OP.mult)
                        nc.vector.tensor_scalar(
                            out=Gn[:, :], in0=Gn[:, :], scalar1=thr[:, 6:7],
                            scalar2=None, op0=OP.subtract)
                        nc.vector.tensor_scalar(
                            out=G0[:, :], in0=V[:, :], scalar1=0.0,
                            scalar2=None, op0=OP.is_ge)
                        nc.vector.tensor_tensor(out=G[:, :], in0=G[:, :],
                                                in1=Gn[:, :], op=OP.subtract)
                        nc.vector.tensor_tensor(out=G[:, :], in0=G0[:, :],
                                                in1=G[:, :], op=OP.mult)
                        nc.vector.tensor_tensor(out=Ofin[:, :], in0=G[:, :],
                                                in1=Gn[:, :], op=OP.add)
                    nc.sync.dma_start(out=O_out[b], in_=Ofin[:, 0:Lp])

    nc.compile()
    return nc


def _get_nc(fast, unit):
    key = (fast, unit)
    if key not in _CACHE:
        _CACHE[key] = _build(fast, unit)
    return _CACHE[key]


def kernel(I, W, threshold_plus, threshold_minus, threshold_plus_sign,
           threshold_minus_sign):
    from concourse.bass_utils import run_bass_kernel_spmd

    tp = np.asarray(threshold_plus, dtype=np.float32)
    tm = np.asarray(threshold_minus, dtype=np.float32)
    ps = np.asarray(threshold_plus_sign, dtype=np.float32)
    ms = np.asarray(threshold_minus_sign, dtype=np.float32)
    I = np.ascontiguousarray(np.asarray(I, dtype=np.float32))
    W = np.asarray(W, dtype=np.float32)
    # [Cout, Cin, K] -> [Cin, K*Cout] so the on-device weight DMA is
    # a contiguous read (layout prep only; all math stays on device)
    Wt = np.ascontiguousarray(
        W.transpose(1, 2, 0).reshape(Cin, K * Cout))

    fast = np.array_equal(tp, tm) and np.array_equal(ps, ms)
    unit = fast and bool(np.all(ps == 1.0))

    thr = np.zeros((Cout, 8), dtype=np.float32)
    thr[:, 0] = -tp
    thr[:, 1] = tp
    thr[:, 2] = tm
    thr[:, 3] = 2.0 * ps
    thr[:, 4] = ps
    thr[:, 5] = 2.0 * ms
    thr[:, 6] = ms

    nc = _get_nc(fast, unit)
    in_maps = [
        {"I": I[c * BPC:(c + 1) * BPC], "W": Wt, "thr": thr}
        for c in range(NCORES)
    ]
    res = run_bass_kernel_spmd(nc, in_maps, list(range(NCORES)))
    out = np.concatenate(
        [np.asarray(r["O"]).astype(np.float32) for r in res.results], axis=0)
    return out


# revision 21
# speedup vs baseline: 6.2589x; 1.0482x over previous
"""Binary conv1d + maxpool + per-channel threshold, Trainium2 Bass kernel.

Problem (hardcoded shapes):
  I:  [64, 64, 16384] f32   -> pad L by (3,3) with -1.0, sign()
  W:  [128, 64, 7]    f32   -> sign()
  conv1d (VALID over padded) -> [64, 128, 16384]
  maxpool1d(k=7, s=2)        -> [64, 128, 8189]
  per-channel threshold      -> +-sign outputs

Sharding: data-parallel over batch, 8 batches per core on 8 cores.

Device algorithm per core (8 batches, as 4 pairs):
  - ScalarE binarizes (Sign, +-1 bf16) into parity-split tiles (even /
    odd padded columns separately) so matmul rhs slices are contiguous.
    A batch pair is stacked on the 128 partitions (batch 2p on 0:64,
    2p+1 on 64:128) and one full-width ACT pass covers both.
  - Conv: 7 accumulating matmuls per output-parity, K=64 contract.  The
    two batches run CONCURRENTLY on the two PE array halves via row
    tiling; even and odd conv columns accumulate into separate PSUM
    tiles so pool ops never need two PSUM operands.
  - ScalarE evacuates even conv columns (Copy, PSUM->SBUF bf16).
  - DVE pool stage 1: T[i] = max(ce[i], psum_odd[i], ce[i+1]) as two
    non-in-place even-width tensor_tensor maxes (16-bit 2x mode).
  - Pool tail per batch: out[l] = max(T[l], T[l+1], T[l+2]) (2 DVE ops).
  - Threshold out = ps*sign(pooled - tp): split between ACT (Sign with
    per-channel bias) and DVE (is_gt chain) by a balance knob.
  - GpSimd is intentionally idle: its tensor ops measured ~19 cyc/elem
    AND stall concurrent DVE work via the shared SBUF port lock.
"""

import numpy as np

B, Cin, L = 64, 64, 16384
Cout, K = 128, 7
PAD = 3
LPAD = L + 2 * PAD          # 16390
Lp = (L - 7) // 2 + 1       # 8189
NT = Lp + 3                 # 8192 T-buffer slots (8191 real + 1 garbage)
NCORES = 8
BPC = B // NCORES           # 8 batches per core
PAIRS = BPC // 2            # 4
NPAR = LPAD // 2            # 8195 entries in each parity tile

GROUP = 1024                # conv cols per group (512 even + 512 odd)
GSTRIDE = GROUP - 2
NGROUPS = 16                # cover T[0:8176)
TAIL_S = 16352
TAIL_W = 32
SIGN_CHUNK = 4096

# batches whose threshold runs on ScalarE (Sign+bias); rest on DVE
ACT_THRESH_BATCHES = 4
# every EVAC_DVE_MOD-th PSUM->SBUF evacuation copy runs on DVE instead of
# ScalarE (balances the two engines; ScalarE is otherwise the busiest)
EVAC_DVE_MOD = 3

_CACHE = {}


def _build(fast: bool, unit: bool):
    import concourse.mybir as mybir
    from concourse import bacc
    from concourse.tile import TileContext

    f32 = mybir.dt.float32
    bf16 = mybir.dt.bfloat16
    AF = mybir.ActivationFunctionType
    OP = mybir.AluOpType

    nc = bacc.Bacc()
    I_in = nc.declare_dram_parameter("I", [BPC, Cin, L], f32, isOutput=False)
    # W is passed host-transposed to [Cin, K, Cout] so this DMA reads
    # long contiguous runs (the natural [Cout, Cin, K] layout degenerates
    # into 4-byte strided descriptors).
    W_in = nc.declare_dram_parameter("W", [Cin, K * Cout], f32,
                                     isOutput=False)
    thr_in = nc.declare_dram_parameter("thr", [Cout, 8], f32, isOutput=False)
    O_out = nc.declare_dram_parameter("O", [BPC, Cout, Lp], bf16, isOutput=True)

    with TileContext(nc) as tc:
        with (
            tc.tile_pool(name="wpool", bufs=1) as wpool,
            tc.tile_pool(name="spool", bufs=2 if fast else 1) as spool,
            tc.tile_pool(name="fpool", bufs=2) as fpool,
            tc.tile_pool(name="tpool", bufs=2) as tpool,
            tc.tile_pool(name="vpool", bufs=2 if fast else 1) as vpool,
            tc.tile_pool(name="opool", bufs=2) as opool,
            tc.tile_pool(name="gpool", bufs=1) as gpool,
            tc.tile_pool(name="cepool", bufs=4) as cepool,
            tc.tile_pool(name="rpool", bufs=2) as rpool,
            tc.tile_pool(name="pspool", bufs=8, space="PSUM") as pspool,
        ):
            # ---- weight prep: sign(W) as {1,-1} bf16, layout [ci, k*128+co]
            wf = wpool.tile([128, K * Cout], f32, tag="wf")
            nc.sync.dma_start(out=wf[0:64, :], in_=W_in[:])
            nc.sync.dma_start(out=wf[64:128, :], in_=W_in[:])
            wb = wpool.tile([128, K * Cout], bf16, tag="wb")
            nc.scalar.activation(out=wb[:, :], in_=wf[:, :], func=AF.Sign)

            # ---- thresholds [128, 8] f32 (col0 = -tp for ACT bias, etc.)
            thr = wpool.tile([128, 8], f32, tag="thr")
            nc.sync.dma_start(out=thr[:, :], in_=thr_in[:])

            groups = [(g * GSTRIDE, GROUP, g * (GROUP // 2 - 1))
                      for g in range(NGROUPS)]
            groups.append((TAIL_S, TAIL_W, NGROUPS * (GROUP // 2 - 1)))

            batch_idx = 0
            for p in range(PAIRS):
                # ---- binarize +-1, parity-split, batch pair stacked
                SEt = spool.tile([128, NPAR], bf16, tag="SE")
                SOt = spool.tile([128, NPAR], bf16, tag="SO")
                for c0 in range(0, L, SIGN_CHUNK):
                    F = fpool.tile([128, SIGN_CHUNK], f32, tag="F")
                    nc.sync.dma_start(
                        out=F[:, :],
                        in_=I_in[2 * p:2 * p + 2, :, c0:c0 + SIGN_CHUNK]
                        .rearrange("b ci l -> (b ci) l"))
                    Fv = F[:].rearrange("p (n two) -> p n two", two=2)
                    half = SIGN_CHUNK // 2
                    # input col i lands at padded col i+3: odd i -> even pad
                    nc.scalar.activation(
                        out=SEt[:, c0 // 2 + 2:c0 // 2 + 2 + half],
                        in_=Fv[:, :, 1], func=AF.Sign)
                    nc.scalar.activation(
                        out=SOt[:, c0 // 2 + 1:c0 // 2 + 1 + half],
                        in_=Fv[:, :, 0], func=AF.Sign)
                # padding -> -1
                nc.vector.memset(SEt[:, 0:2], -1.0)
                nc.vector.memset(SEt[:, NPAR - 1:NPAR], -1.0)
                nc.vector.memset(SOt[:, 0:1], -1.0)
                nc.vector.memset(SOt[:, NPAR - 2:NPAR], -1.0)

                # ---- conv + fused pool stage 1 into T buffers
                Tlo = tpool.tile([128, NT], bf16, tag="T")
                Thi = tpool.tile([128, NT], bf16, tag="T")

                def rhs(par, half, s, tap, n):
                    # conv col j = s + 2i (+1 if par odd), tap k:
                    # padded[j + k] column index
                    if par == 0:
                        src, n0 = (SEt, (s + tap) // 2) if tap % 2 == 0 \
                            else (SOt, (s + tap - 1) // 2)
                    else:
                        src, n0 = (SOt, (s + tap) // 2) if tap % 2 == 0 \
                            else (SEt, (s + tap + 1) // 2)
                    return src[64 * half:64 * (half + 1), n0:n0 + n]

                for gi, (s, w, t0) in enumerate(groups):
                    h = w // 2
                    pse = [pspool.tile([128, h], f32, tag="ps",
                                       name=f"pse{i}_{p}_{s}")
                           for i in range(2)]
                    pso = [pspool.tile([128, h], f32, tag="ps",
                                       name=f"pso{i}_{p}_{s}")
                           for i in range(2)]
                    for tap in range(K):
                        st = (tap == 0)
                        sp = (tap == K - 1)
                        for half in range(2):
                            lw = wb[64 * half:64 * (half + 1),
                                    tap * Cout:(tap + 1) * Cout]
                            nc.tensor.matmul(
                                pse[half][:, 0:h], lw, rhs(0, half, s, tap, h),
                                start=st, stop=sp)
                            nc.tensor.matmul(
                                pso[half][:, 0:h], lw, rhs(1, half, s, tap, h),
                                start=st, stop=sp)
                    for (half, Tb) in ((0, Tlo), (1, Thi)):
                        CE = cepool.tile([128, 520], bf16, tag="CE")
                        if (2 * gi + half) % EVAC_DVE_MOD == 0:
                            nc.vector.tensor_copy(out=CE[:, 0:h],
                                                  in_=pse[half][:, 0:h])
                        else:
                            nc.scalar.activation(out=CE[:, 0:h],
                                                 in_=pse[half][:, 0:h],
                                                 func=AF.Copy)
                        nc.vector.memset(CE[:, h:h + 2], 0.0)
                        R = rpool.tile([128, 512], bf16, tag="R")
                        nc.vector.tensor_tensor(
                            out=R[:, 0:h], in0=CE[:, 0:h],
                            in1=pso[half][:, 0:h], op=OP.max)
                        nc.vector.tensor_tensor(
                            out=Tb[:, t0:t0 + h], in0=R[:, 0:h],
                            in1=CE[:, 1:h + 1], op=OP.max)

                # ---- pool tail + threshold + store, per batch
                for (b, Tb) in ((2 * p, Tlo), (2 * p + 1, Thi)):
                    on_act = batch_idx >= (BPC - ACT_THRESH_BATCHES)
                    batch_idx += 1
                    V = vpool.tile([128, Lp + 1], bf16, tag="V")
                    Ofin = opool.tile([128, Lp + 1], bf16, tag="Ofin")
                    nc.vector.tensor_tensor(out=V[:, 0:Lp + 1],
                                            in0=Tb[:, 0:Lp + 1],
                                            in1=Tb[:, 1:Lp + 2], op=OP.max)
                    nc.vector.tensor_tensor(out=V[:, 0:Lp + 1],
                                            in0=V[:, 0:Lp + 1],
                                            in1=Tb[:, 2:Lp + 3], op=OP.max)
                    if fast:
                        if on_act:
                            # sign(pooled - tp), times ps if needed
                            nc.scalar.activation(out=Ofin[:, :], in_=V[:, :],
                                                 func=AF.Sign,
                                                 bias=thr[:, 0:1])
                            if not unit:
                                nc.vector.tensor_scalar(
                                    out=Ofin[:, :], in0=Ofin[:, :],
                                    scalar1=thr[:, 4:5], scalar2=None,
                                    op0=OP.mult)
                        else:
                            s2 = 2.0 if unit else thr[:, 3:4]
                            s3 = 1.0 if unit else thr[:, 4:5]
                            nc.vector.tensor_scalar(
                                out=V[:, :], in0=V[:, :], scalar1=thr[:, 1:2],
                                scalar2=s2, op0=OP.is_gt, op1=OP.mult)
                            nc.vector.tensor_scalar(
                                out=Ofin[:, :], in0=V[:, :], scalar1=s3,
                                scalar2=None, op0=OP.subtract)
                    else:
                        G = gpool.tile([128, Lp + 1], bf16, tag="G")
                        Gn = gpool.tile([128, Lp + 1], bf16, tag="Gn")
                        G0 = gpool.tile([128, Lp + 1], bf16, tag="G0")
                        # pos branch: {ps, -ps}
                        nc.vector.tensor_scalar(
                            out=G[:, :], in0=V[:, :], scalar1=thr[:, 1:2],
                            scalar2=thr[:, 3:4], op0=OP.is_gt, op1=OP.mult)
                        nc.vector.tensor_scalar(
                            out=G[:, :], in0=G[:, :], scalar1=thr[:, 4:5],
                            scalar2=None, op0=OP.subtract)
                        # neg branch: {ms, -ms}
                        nc.vector.tensor_scalar(
                            out=Gn[:, :], in0=V[:, :], scalar1=thr[:, 2:3],
                            scalar2=thr[:, 5:6], op0=OP.is_gt, op1=OP.mult)
                        nc.vector.tensor_scalar(
                            out=Gn[:, :], in0=Gn[:, :], scalar1=thr[:, 6:7],
                            scalar2=None, op0=OP.subtract)
                        nc.vector.tensor_scalar(
                            out=G0[:, :], in0=V[:, :], scalar1=0.0,
                            scalar2=None, op0=OP.is_ge)
                        nc.vector.tensor_tensor(out=G[:, :], in0=G[:, :],
                                                in1=Gn[:, :], op=OP.subtract)
                        nc.vector.tensor_tensor(out=G[:, :], in0=G0[:, :],
                                                in1=G[:, :], op=OP.mult)
                        nc.vector.tensor_tensor(out=Ofin[:, :], in0=G[:, :],
                                                in1=Gn[:, :], op=OP.add)
                    nc.sync.dma_start(out=O_out[b], in_=Ofin[:, 0:Lp])

    nc.compile()
    return nc


def _get_nc(fast, unit):
    key = (fast, unit)
    if key not in _CACHE:
        _CACHE[key] = _build(fast, unit)
    return _CACHE[key]


def kernel(I, W, threshold_plus, threshold_minus, threshold_plus_sign,
           threshold_minus_sign):
    from concourse.bass_utils import run_bass_kernel_spmd

    tp = np.asarray(threshold_plus, dtype=np.float32)
    tm = np.asarray(threshold_minus, dtype=np.float32)
    ps = np.asarray(threshold_plus_sign, dtype=np.float32)
    ms = np.asarray(threshold_minus_sign, dtype=np.float32)
    I = np.ascontiguousarray(np.asarray(I, dtype=np.float32))
    W = np.asarray(W, dtype=np.float32)
    # [Cout, Cin, K] -> [Cin, K*Cout] so the on-device weight DMA is
    # a contiguous read (layout prep only; all math stays on device)
    Wt = np.ascontiguousarray(
        W.transpose(1, 2, 0).reshape(Cin, K * Cout))

    fast = np.array_equal(tp, tm) and np.array_equal(ps, ms)
    unit = fast and bool(np.all(ps == 1.0))

    thr = np.zeros((Cout, 8), dtype=np.float32)
    thr[:, 0] = -tp
    thr[:, 1] = tp
    thr[:, 2] = tm
    thr[:, 3] = 2.0 * ps
    thr[:, 4] = ps
    thr[:, 5] = 2.0 * ms
    thr[:, 6] = ms

    nc = _get_nc(fast, unit)
    in_maps = [
        {"I": I[c * BPC:(c + 1) * BPC], "W": Wt, "thr": thr}
        for c in range(NCORES)
    ]
    res = run_bass_kernel_spmd(nc, in_maps, list(range(NCORES)))
    out = np.concatenate(
        [np.asarray(r["O"]).astype(np.float32) for r in res.results], axis=0)
    return out
